# revision 2
# baseline (speedup 1.0000x reference)
"""Two-phase Bass/Tile kernels for the Contrast5 loss (SPMD, 8 cores x 3 batches).

Kernel A: unc = sum_c pred*ln(pred+1e-6) -> per-partition top-8 values+indices
          (1024 candidates per batch; the true top-5 is always a subset).
Kernel B: given gathered unit-feature candidates (64, 45), compute the loss partial.
Host in between: top-5-of-1024 selection + 360-vector gather from proj.
"""

import sys
for _p in ("/root/.axon_site/_ro/trn_rl_repo", "/opt/trn_rl_repo"):
    if _p not in sys.path:
        sys.path.append(_p)
import numpy as np
import concourse.bass as bass
import concourse.bacc as bacc
import concourse.mybir as mybir
import concourse.tile as tile

F32 = mybir.dt.float32
U32 = mybir.dt.uint32
AF = mybir.ActivationFunctionType
OP = mybir.AluOpType
AX = mybir.AxisListType

B_LOC = 3
C = 4
HW = 65536
D = 64
S = 5
NI = 3
TAU = 0.07
EPS_LOG = 1e-6
EPS_DEN = 1e-8
NCORES = 8


def build_nc_a():
    nc = bacc.Bacc("TRN2", target_bir_lowering=False, debug=False)
    pred_in = nc.dram_tensor("pred", [B_LOC, C, HW], F32, kind="ExternalInput")
    m8_out = nc.dram_tensor("m8", [128, B_LOC * 8], F32, kind="ExternalOutput")
    i8_out = nc.dram_tensor("i8", [128, B_LOC * 8], U32, kind="ExternalOutput")

    with tile.TileContext(nc) as tc:
        with tc.tile_pool(name="sb", bufs=3) as pool, tc.tile_pool(
            name="cst", bufs=1
        ) as cpool:
            eps_bias = cpool.tile([128, 1], F32, tag="eps_bias")
            nc.vector.memset(eps_bias[:], EPS_LOG)
            m8all = cpool.tile([128, B_LOC * 8], F32, tag="m8all")
            i8all = cpool.tile([128, B_LOC * 8], U32, tag="i8all")
            for b in range(B_LOC):
                predt = pool.tile([128, C * 512], F32, tag="pred")
                for c in range(C):
                    nc.sync.dma_start(
                        out=predt[:, c * 512 : (c + 1) * 512],
                        in_=pred_in[b, c].rearrange("(p x) -> p x", p=128),
                    )
                logt = pool.tile([128, C * 512], F32, tag="logt")
                nc.scalar.activation(
                    out=logt[:], in_=predt[:], func=AF.Ln, bias=eps_bias[:], scale=1.0
                )
                prodt = pool.tile([128, C * 512], F32, tag="prodt")
                nc.vector.tensor_tensor(
                    out=prodt[:], in0=predt[:], in1=logt[:], op=OP.mult
                )
                unct = pool.tile([128, 512], F32, tag="unct")
                nc.vector.reduce_sum(
                    out=unct[:],
                    in_=prodt[:].rearrange("p (c x) -> p x c", c=C),
                    axis=AX.X,
                )
                nc.vector.max(m8all[:, b * 8 : (b + 1) * 8], unct[:])
                nc.vector.max_index(
                    i8all[:, b * 8 : (b + 1) * 8],
                    m8all[:, b * 8 : (b + 1) * 8],
                    unct[:],
                )
            nc.sync.dma_start(out=m8_out[:], in_=m8all[:])
            nc.sync.dma_start(out=i8_out[:], in_=i8all[:])
    nc.compile()
    return nc


def build_nc_b():
    nc = bacc.Bacc("TRN2", target_bir_lowering=False, debug=False)
    psel_in = nc.dram_tensor("psel", [D, B_LOC * NI * S], F32, kind="ExternalInput")
    ones_row_in = nc.dram_tensor("ones_row", [1, D], F32, kind="ExternalInput")
    ones_col_in = nc.dram_tensor("ones_col", [D, 1], F32, kind="ExternalInput")
    posmask_in = nc.dram_tensor("posmask", [S, NI * S], F32, kind="ExternalInput")
    diag_in = nc.dram_tensor("diag5", [S, S], F32, kind="ExternalInput")
    out_dram = nc.dram_tensor("out", [1, 1], F32, kind="ExternalOutput")

    with tile.TileContext(nc) as tc:
        with (
            tc.tile_pool(name="sb", bufs=2) as pool,
            tc.tile_pool(name="cst", bufs=1) as cpool,
            tc.tile_pool(name="ps", bufs=1, space="PSUM") as pp,
        ):
            onesr = cpool.tile([1, D], F32, tag="onesr")
            nc.sync.dma_start(out=onesr[:], in_=ones_row_in[:])
            onesc = cpool.tile([D, 1], F32, tag="onesc")
            nc.sync.dma_start(out=onesc[:], in_=ones_col_in[:])
            posmask = cpool.tile([S, NI * S], F32, tag="posmask")
            nc.sync.dma_start(out=posmask[:], in_=posmask_in[:])
            diag5 = cpool.tile([S, S], F32, tag="diag5")
            nc.sync.dma_start(out=diag5[:], in_=diag_in[:])
            psel = cpool.tile([D, B_LOC * NI * S], F32, tag="psel")
            nc.sync.dma_start(out=psel[:], in_=psel_in[:])

            sq = cpool.tile([D, B_LOC * NI * S], F32, tag="sq")
            nc.scalar.activation(out=sq[:], in_=psel[:], func=AF.Square)
            nrm_ps = pp.tile([1, B_LOC * NI * S], F32, tag="nrm")
            nc.tensor.matmul(nrm_ps[:], lhsT=onesc[:], rhs=sq[:], start=True, stop=True)
            nrm_sq = cpool.tile([1, B_LOC * NI * S], F32, tag="nrm_sq")
            nc.scalar.activation(out=nrm_sq[:], in_=nrm_ps[:], func=AF.Sqrt)
            rinv = cpool.tile([1, B_LOC * NI * S], F32, tag="rinv")
            nc.vector.reciprocal(out=rinv[:], in_=nrm_sq[:])
            rb_ps = pp.tile([D, B_LOC * NI * S], F32, tag="rb")
            nc.tensor.matmul(rb_ps[:], lhsT=onesr[:], rhs=rinv[:], start=True, stop=True)
            phat = cpool.tile([D, B_LOC * NI * S], F32, tag="phat")
            nc.vector.tensor_tensor(out=phat[:], in0=psel[:], in1=rb_ps[:], op=OP.mult)

            lgall = cpool.tile([S, B_LOC], F32, tag="lgall")
            for b in range(B_LOC):
                xb = phat[:, b * NI * S : (b + 1) * NI * S]
                g_ps = pp.tile([NI * S, NI * S], F32, tag="g")
                nc.tensor.matmul(g_ps[:], lhsT=xb, rhs=xb, start=True, stop=True)
                tmp = pool.tile([S, NI * S], F32, tag="tmp")
                nc.vector.tensor_tensor(
                    out=tmp[:], in0=g_ps[0:S, :], in1=posmask[:], op=OP.mult
                )
                pos_sim = pool.tile([S, 1], F32, tag="pos_sim")
                nc.vector.reduce_sum(out=pos_sim[:], in_=tmp[:], axis=AX.X)
                pl = pool.tile([S, 1], F32, tag="pl")
                nc.scalar.activation(
                    out=pl[:], in_=pos_sim[:], func=AF.Exp, scale=1.0 / TAU
                )
                em = pool.tile([S, S], F32, tag="em")
                nc.scalar.activation(
                    out=em[:], in_=g_ps[0:S, 0:S], func=AF.Exp, scale=1.0 / TAU
                )
                cs_ps = pp.tile([S, 1], F32, tag="cs")
                nc.tensor.matmul(
                    cs_ps[:], lhsT=em[:], rhs=onesc[0:S, :], start=True, stop=True
                )
                tmp2 = pool.tile([S, S], F32, tag="tmp2")
                nc.vector.tensor_tensor(
                    out=tmp2[:], in0=em[:], in1=diag5[:], op=OP.mult
                )
                diag = pool.tile([S, 1], F32, tag="diag")
                nc.vector.reduce_sum(out=diag[:], in_=tmp2[:], axis=AX.X)
                neg = pool.tile([S, 1], F32, tag="neg")
                nc.vector.scalar_tensor_tensor(
                    out=neg[:], in0=cs_ps[:], scalar=EPS_DEN, in1=diag[:],
                    op0=OP.add, op1=OP.subtract,
                )
                den = pool.tile([S, 1], F32, tag="den")
                nc.vector.tensor_tensor(out=den[:], in0=neg[:], in1=pl[:], op=OP.add)
                lg = pool.tile([S, 1], F32, tag="lg")
                nc.scalar.activation(out=lg[:], in_=den[:], func=AF.Ln)
                nc.vector.scalar_tensor_tensor(
                    out=lgall[:, b : b + 1], in0=pos_sim[:], scalar=-1.0 / TAU,
                    in1=lg[:], op0=OP.mult, op1=OP.add,
                )
            row_ps = pp.tile([1, B_LOC], F32, tag="row")
            nc.tensor.matmul(
                row_ps[:], lhsT=onesc[0:S, :], rhs=lgall[:], start=True, stop=True
            )
            tot = pool.tile([1, 1], F32, tag="tot")
            nc.vector.reduce_sum(out=tot[:], in_=row_ps[:], axis=AX.X)
            outt = pool.tile([1, 1], F32, tag="outt")
            nc.vector.tensor_scalar_mul(outt[:], tot[:], 1.0 / S)
            nc.sync.dma_start(out=out_dram[:], in_=outt[:])
    nc.compile()
    return nc


def host_constants_b():
    posmask = np.zeros((S, NI * S), np.float32)
    for s in range(S):
        posmask[s, S + s] = 1.0
        posmask[s, 2 * S + s] = 1.0
    return {
        "ones_row": np.ones((1, D), np.float32),
        "ones_col": np.ones((D, 1), np.float32),
        "posmask": posmask,
        "diag5": np.eye(S, dtype=np.float32),
    }


def select_and_gather(m8, i8, proj_flat_core):
    """m8/i8 (128, 24) per core; proj_flat_core: list of NI arrays (B_LOC*D*HW,).
    Returns psel (64, 45) f32 and chosen pixel indices (B_LOC, S)."""
    psel = np.empty((D, B_LOC * NI * S), np.float32)
    chosen = np.empty((B_LOC, S), np.int64)
    for b in range(B_LOC):
        vals = m8[:, b * 8 : (b + 1) * 8].ravel()
        offs = i8[:, b * 8 : (b + 1) * 8].ravel()
        top = np.argsort(-vals, kind="stable")[:S]
        part = top // 8
        hw = part * 512 + offs[top]
        chosen[b] = hw
        for i in range(NI):
            pj = proj_flat_core[i].reshape(B_LOC, D, HW)
            psel[:, b * NI * S + i * S : b * NI * S + (i + 1) * S] = pj[b][:, hw]
    return psel, chosen


def shard_pred(pred):
    pred_r = np.ascontiguousarray(pred.reshape(24, C, HW))
    return [
        {"pred": pred_r[c * B_LOC : (c + 1) * B_LOC]} for c in range(NCORES)
    ]


# ---------------------------------------------------------------------------
# Harness entry point: kernel(**inputs) -> full-shape output (scalar f32).
# Two NEFF launches on 8 NeuronCores: (A) uncertainty + per-partition top-8
# candidates, (B) loss from host-gathered top-5 feature vectors. The host in
# between only selects top-5-of-1024 per batch and slices 360 tiny vectors.
# ---------------------------------------------------------------------------
from concourse.bass_utils import run_bass_kernel_spmd

_CACHE = {}


def _get_programs():
    if "a" not in _CACHE:
        _CACHE["a"] = build_nc_a()
        _CACHE["b"] = build_nc_b()
    return _CACHE["a"], _CACHE["b"]


def kernel(pred, proj, mask, pseudo_label, idx, sample_num):
    assert int(idx) == 0 and int(sample_num) == S
    pred = np.ascontiguousarray(np.asarray(pred, dtype=np.float32))
    proj = np.asarray(proj, dtype=np.float32)
    nc_a, nc_b = _get_programs()
    core_ids = list(range(NCORES))

    res_a = run_bass_kernel_spmd(nc_a, shard_pred(pred), core_ids=core_ids)

    consts = host_constants_b()
    in_maps_b = []
    for core in range(NCORES):
        b0 = core * B_LOC
        proj_core = [proj[i, b0 : b0 + B_LOC].reshape(-1) for i in range(NI)]
        psel, _ = select_and_gather(
            res_a.results[core]["m8"], res_a.results[core]["i8"], proj_core
        )
        in_maps_b.append({"psel": psel, **consts})

    res_b = run_bass_kernel_spmd(nc_b, in_maps_b, core_ids=core_ids)
    partials = np.array(
        [r["out"].ravel()[0] for r in res_b.results], dtype=np.float32
    )
    return np.float32(partials.sum() / 24.0).reshape(())
